# revision 1
# baseline (speedup 1.0000x reference)
# Trainium2 Bass kernel for the DVAE encoder (nn_DVAE_24850680775463).
#
# Sharding: pure data-parallel. B=1024 graphs -> 8 cores x 128 graphs.
#
# Feature-major design: the hidden state lives as [128 feat-part, 4*128]
# (feature chunk on partitions, graphs on the free dim). GRU gate matmuls
# run "flipped" (weight tile stationary, hidden state moving), so gate
# pre-activations come out of PSUM already feature-major and NO transposes
# are needed anywhere. Per-feature biases enter via K=1 matmuls (bias row
# stationary, ones row moving). The adjacency-weighted message is computed
# directly feature-major with gm chunks stationary against diag(adj).
# Elementwise runs in column halves so it pipelines against the PE stream,
# with the z-path offloaded to GpSimd.

import os
import numpy as np

import concourse.bass as bass
import concourse.tile as tile
from concourse import bacc, mybir
from concourse.bass_utils import run_bass_kernel_spmd

AF = mybir.ActivationFunctionType
ALU = mybir.AluOpType
F32 = mybir.dt.float32

NCORES = 8
B, NV, NVT, FS, HS, NZ = 1024, 16, 16, 32, 512, 64
P = B // NCORES            # 128 graphs per core
G3 = 3 * HS                # 1536
K1 = NVT + 1               # 17  (one-hot + ones row)
K2 = FS + 1                # 33  (params + ones row)
KC = HS // 128             # 4 feature chunks of the hidden dim
HHALF = HS // 2            # 256

MMDT = {"f32r": mybir.dt.float32r, "f32": mybir.dt.float32,
        "bf16": mybir.dt.bfloat16}[os.environ.get("DVAE_MMDT", "bf16")]
DEBUG = os.environ.get("DVAE_DEBUG", "0") == "1"
NO_GPSIMD = os.environ.get("DVAE_NO_GPSIMD", "0") == "1"


def build_bass():
    nc = bacc.Bacc("TRN2", target_bir_lowering=False, debug=False)

    def inp(name, shape, dt=None):
        return nc.dram_tensor(name, shape, dt or MMDT,
                              kind="ExternalInput").ap()

    d = {
        "wht_t": inp("wht_t", [128, KC * G3]),
        "wht_p": inp("wht_p", [128, KC * G3]),
        "w1x":   inp("w1x",   [K1, G3]),
        "w2x":   inp("w2x",   [K2, G3]),
        "xt1":   inp("xt1",   [K1, NV * P]),
        "xp1":   inp("xp1",   [K2, NV * P]),
        "xni_t": inp("xni_t", [128, NV * HS]),
        "xni_p": inp("xni_p", [128, NV * HS]),
        "bhnf":  inp("bhnf",  [128, 8], F32),
        "wgm":   inp("wgm",   [128, KC * 2 * HS]),
        "bgm":   inp("bgm",   [NV, 2 * HS]),
        "vsel":  inp("vsel",  [NV, NV * P]),
        "adjt":  inp("adjt",  [P, NV * NV], F32),
        "wfc":   inp("wfc",   [128, KC * 2 * NZ]),
        "bfc":   inp("bfc",   [1, 2 * NZ]),
        "eye":   inp("eye",   [128, 128], F32),
        "ones1": inp("ones1", [1, 128]),
    }
    out_ap = nc.dram_tensor("out", [P, 2 * NZ], mybir.dt.float32, kind="ExternalOutput").ap()

    with tile.TileContext(nc) as tc:
        _body(tc, d, out_ap)
    nc.compile()
    return nc


def _body(tc, d, out_ap):
    nc = tc.nc
    from contextlib import ExitStack
    with ExitStack() as ctx:
        wp = ctx.enter_context(tc.tile_pool(name="w", bufs=1))
        sp = ctx.enter_context(tc.tile_pool(name="s", bufs=2))
        dgp = ctx.enter_context(tc.tile_pool(name="dg", bufs=16))
        gmc = ctx.enter_context(tc.tile_pool(name="gmc", bufs=1))
        ps_g = ctx.enter_context(tc.tile_pool(name="psg", bufs=4, space="PSUM"))
        ps_h = ctx.enter_context(tc.tile_pool(name="psh", bufs=1, space="PSUM"))
        ps_m = ctx.enter_context(tc.tile_pool(name="psm", bufs=1, space="PSUM"))

        # ---- persistent weights / constants (load order = need order;
        # spread across engine DMA queues so big loads go in parallel) ----
        order = ["ones1", "xt1", "w1x", "bhnf", "eye", "adjt", "w2x", "xp1",
                 "xni_t", "xni_p", "vsel", "bgm", "wgm", "wht_t", "wht_p",
                 "wfc", "bfc"]
        qs = [nc.sync, nc.scalar, nc.gpsimd]
        W = {}
        for i, (name, ap) in enumerate(
                sorted(d.items(), key=lambda kv: order.index(kv[0]))):
            t = wp.tile(list(ap.shape), ap.dtype, tag=name)
            qs[i % len(qs)].dma_start(t[:], ap[:, :])
            W[name] = t

        wht = {0: W["wht_t"], 1: W["wht_p"]}
        wx = {0: W["w1x"], 1: W["w2x"]}
        xs = {0: W["xt1"], 1: W["xp1"]}
        kx = {0: K1, 1: K2}
        eye = W["eye"]
        adjt = W["adjt"]
        ones1 = W["ones1"]

        gm_sb = []          # cached gate*mapped per vertex, [P, HS] batch-major

        def alloc_banks():
            return [ps_g.tile([128, HS], F32, tag="g", name=f"bank{i}")
                    for i in range(3)]

        def gru_gate_regions(g, v, banks, hT, ms):
            """R/NH/Z gate regions, each accumulation group emitted
            contiguously — interleaved multi-mm groups on column slices of
            one bank corrupt PSUM (bank-wide clear on start=True)."""
            R, Z, NH = banks
            K = kx[g]
            xr = xs[g][:K, v * P:(v + 1) * P]
            xw = wx[g]
            w = wht[g]
            nohid = hT is None
            for go, bank in ((0, R), (2, NH), (1, Z)):
                for m in ms:
                    sl = slice(m * 128, (m + 1) * 128)
                    if go != 2:
                        nc.tensor.matmul(
                            bank[:, sl],
                            xw[:K, go * HS + m * 128:go * HS + (m + 1) * 128],
                            xr, start=True, stop=nohid)
                    if nohid:
                        continue
                    for k in range(KC):
                        nc.tensor.matmul(
                            bank[:, sl],
                            w[:, k * G3 + go * HS + m * 128:k * G3 + go * HS + (m + 1) * 128],
                            hT[:, k * 128:(k + 1) * 128],
                            start=(go == 2 and k == 0), stop=k == KC - 1)

        def halves(t):
            return t[:, 0:HHALF], t[:, HHALF:HS]

        def gru_ew_wave1(g, banks, h_sb, tags):
            """Bank-draining wave: every read of the 3 PSUM banks is emitted
            here, so ring slots may be safely re-started right after."""
            R, Z, NH = banks
            r = sp.tile([128, HS], MMDT, tag=tags + "r")
            z = sp.tile([128, HS], MMDT, tag=tags + "z")
            rhn = sp.tile([128, HS], MMDT, tag=tags + "rhn")
            for hf in range(2):
                sl = slice(hf * HHALF, (hf + 1) * HHALF)
                nc.scalar.activation(r[:, sl], R[:, sl], AF.Sigmoid)
                if h_sb is not None:
                    nc.scalar.activation(z[:, sl], Z[:, sl], AF.Sigmoid)
                else:
                    # z slot holds zc = 1 - z directly
                    nc.scalar.activation(z[:, sl], Z[:, sl], AF.Sigmoid,
                                         scale=-1.0)
            for c in range(KC):
                sl = slice(c * 128, (c + 1) * 128)
                bs = W["bhnf"][:, g * KC + c:g * KC + c + 1]
                if h_sb is not None:
                    # rhn = (NH + bhn) * r, bhn per-partition within chunk
                    nc.vector.scalar_tensor_tensor(rhn[:, sl], NH[:, sl], bs,
                                                   r[:, sl], ALU.add, ALU.mult)
                else:
                    nc.vector.tensor_scalar_mul(rhn[:, sl], r[:, sl], bs)
            return z, rhn

        def gru_ew_wave2(g, v, h_sb, z, rhn, out_t, tags):
            """Rest of the GRU combine; no PSUM bank reads."""
            n = sp.tile([128, HS], MMDT, tag=tags + "n")
            npre = sp.tile([128, HS], MMDT, tag=tags + "npre")
            xni = W["xni_t"] if g == 0 else W["xni_p"]
            h0, h1 = slice(0, HHALF), slice(HHALF, HS)
            for sl in (h0, h1):
                nc.vector.tensor_add(npre[:, sl], rhn[:, sl],
                                     xni[:, v * HS + sl.start:v * HS + sl.stop])
                nc.scalar.activation(n[:, sl], npre[:, sl], AF.Tanh)
            if h_sb is None:
                nc.vector.tensor_mul(out_t[:, h0], z[:, h0], n[:, h0])
                nc.vector.tensor_mul(out_t[:, h1], z[:, h1], n[:, h1])
                return
            zc = sp.tile([128, HS], MMDT, tag=tags + "zc")
            zh = sp.tile([128, HS], MMDT, tag=tags + "zh")
            zn = sp.tile([128, HS], MMDT, tag=tags + "zn")
            geng = nc.vector if NO_GPSIMD else nc.gpsimd
            for sl in (h0, h1):
                geng.tensor_scalar(zc[:, sl], z[:, sl], -1.0, 1.0,
                                   ALU.mult, ALU.add)
                geng.tensor_mul(zh[:, sl], z[:, sl], h_sb[:, sl])
            nc.vector.tensor_mul(zn[:, h0], zc[:, h0], n[:, h0])
            nc.vector.tensor_add(out_t[:, h0], zn[:, h0], zh[:, h0])
            nc.vector.tensor_mul(zn[:, h1], zc[:, h1], n[:, h1])
            nc.vector.tensor_add(out_t[:, h1], zn[:, h1], zh[:, h1])

        def diag_build(w, u, eng):
            dg = dgp.tile([128, 128], MMDT, tag="diag")
            eng.tensor_scalar_mul(dg[:], eye[:], adjt[:, w * NV + u:w * NV + u + 1])
            return dg

        def dbg_dump(name, t):
            if DEBUG:
                dap = nc.dram_tensor(name, [128, t.shape[1]], t.dtype,
                                     kind="ExternalOutput").ap()
                nc.sync.dma_start(dap[:, :], t[:])

        FILL = [int(x) for x in
                os.environ.get("DVAE_FILL", "5,7").split(",")]

        def fill_pe(k):
            """k dependency-free N=512 matmuls into the Hb scratch bank.
            Keeps the PE HAM activity window dense through elementwise
            phases so real matmuls run at 2.4 GHz instead of 1.2."""
            if k <= 0:
                return
            dum = ps_h.tile([128, HS], F32, tag="Hb", name="dum")
            for _ in range(k):
                nc.tensor.matmul(dum[:], W["wgm"][:, 0:128],
                                 W["wht_t"][:, 0:HS], start=True, stop=True)

        def ha_partials(Ha, dgs, v, cs):
            """H(v+1) partial terms u<v for chunk regions cs (contiguous)."""
            for c in cs:
                for u in range(v):
                    nc.tensor.matmul(
                        Ha[:, c * 128:(c + 1) * 128],
                        gm_sb[u][:, c * 128:(c + 1) * 128],
                        dgs[u][:], start=(u == 0), stop=(u == v - 1))

        # ---------------- step 0 prologue ----------------
        banks_t = alloc_banks()
        H_sb = None          # SBUF feature-major hidden input of GRU_t
        dgs_next = {0: diag_build(1, 0, nc.vector)}

        for v in range(NV):
            v1 = v + 1
            dgs = dgs_next   # diag tiles for message target v+1
            # ---- GRU_t gate regions (x opener + h chunks, contiguous) ----
            gru_gate_regions(0, v, banks_t, H_sb, (0, 1))
            gru_gate_regions(0, v, banks_t, H_sb, (2, 3))

            hv1 = sp.tile([128, HS], MMDT, tag="hv1")
            # drain banks_t fully before re-starting their ring slots
            z_t, rhn_t = gru_ew_wave1(0, banks_t, H_sb, "t")

            banks_p = alloc_banks()
            A_sb = Ha = None
            if v < NV - 1:
                # gate/map PSUM starters for this step (banks free since v-1)
                gatep = ps_m.tile([128, HS], F32, tag="gate")
                mapp = ps_m.tile([128, HS], F32, tag="map")
                vl = W["vsel"][:, v * P:(v + 1) * P]
                nc.tensor.matmul(gatep[:], vl, W["bgm"][:, 0:HS],
                                 start=True, stop=False)
                nc.tensor.matmul(mapp[:], vl, W["bgm"][:, HS:2 * HS],
                                 start=True, stop=False)
                if v >= 1:
                    # H(v+1) partials: first half fills PE during ew_t
                    Ha = ps_h.tile([128, HS], F32, tag="Ha")
                    ha_partials(Ha, dgs, v, (0, 1))
                    A_sb = sp.tile([128, HS], MMDT, tag="Asb")
                    nc.vector.tensor_copy(A_sb[:, 0:HHALF], Ha[:, 0:HHALF])
            fill_pe(max(0, FILL[0] - v // 3) if v > 0 else 0)

            gru_ew_wave2(0, v, H_sb, z_t, rhn_t, hv1, "t")

            # ---- GRU_p gate regions ----
            gru_gate_regions(1, v, banks_p, hv1, (0, 1))
            gru_gate_regions(1, v, banks_p, hv1, (2, 3))
            hv = sp.tile([128, HS], MMDT, tag="hv")
            z_p, rhn_p = gru_ew_wave1(1, banks_p, hv1, "p")

            if v < NV - 1:
                banks_t2 = alloc_banks()
                if v >= 1:
                    # second half of H(v+1) partials fills PE during ew_p
                    ha_partials(Ha, dgs, v, (2, 3))
                    nc.scalar.copy(A_sb[:, HHALF:HS], Ha[:, HHALF:HS])
            fill_pe(max(0, FILL[1] - v // 3) if v > 0 else 0)

            gru_ew_wave2(1, v, hv1, z_p, rhn_p, hv, "p")
            dbg_dump(f"dbg_hv1_{v}", hv1)
            dbg_dump(f"dbg_hv_{v}", hv)
            dbg_dump(f"dbg_zp_{v}", z_p)

            if v < NV - 1:
                # ---- gate/mapper (batch-major; hv chunks stationary) ----
                for c in range(KC):
                    hl = hv[:, c * 128:(c + 1) * 128]
                    last = c == KC - 1
                    nc.tensor.matmul(gatep[:], hl,
                                     W["wgm"][:, c * 2 * HS:c * 2 * HS + HS],
                                     start=False, stop=last)
                    nc.tensor.matmul(mapp[:], hl,
                                     W["wgm"][:, c * 2 * HS + HS:(c + 1) * 2 * HS],
                                     start=False, stop=last)
                gmt = gmc.tile([128, HS], MMDT, tag=f"gm{v}")
                gm_sb.append(gmt)
                H_new = sp.tile([128, HS], MMDT, tag="H", bufs=2)
                gate = sp.tile([128, HS], MMDT, tag="gate")
                Hb = ps_h.tile([128, HS], F32, tag="Hb")
                for hf in range(2):
                    sl = slice(hf * HHALF, (hf + 1) * HHALF)
                    nc.scalar.activation(gate[:, sl], gatep[:, sl], AF.Sigmoid)
                    nc.vector.tensor_mul(gmt[:, sl], gate[:, sl], mapp[:, sl])
                    # final message term: single-mm groups per region
                    for c in (2 * hf, 2 * hf + 1):
                        nc.tensor.matmul(Hb[:, c * 128:(c + 1) * 128],
                                         gmt[:, c * 128:(c + 1) * 128],
                                         dgs[v][:], start=True, stop=True)
                    if A_sb is None:
                        if hf == 0:
                            nc.vector.tensor_copy(H_new[:, sl], Hb[:, sl])
                        else:
                            nc.scalar.copy(H_new[:, sl], Hb[:, sl])
                    else:
                        nc.vector.tensor_add(H_new[:, sl], A_sb[:, sl],
                                             Hb[:, sl])
                # diag tiles for next step's message target (off the chain)
                dgs_next = ({u: diag_build(v + 2, u, nc.vector)
                             for u in range(v + 2)} if v1 < NV - 1 else {})
                dbg_dump(f"dbg_gm_{v}", gmt)
                dbg_dump(f"dbg_H_{v1}", H_new)
                H_sb = H_new
                banks_t = banks_t2
            else:
                # ---- final FC: out = Hg @ Wfc + bfc  (mu | logvar) ----
                fcp = ps_m.tile([128, 2 * NZ], F32, tag="gate")
                nc.tensor.matmul(fcp[:], ones1[0:1, :], W["bfc"][:, :],
                                 start=True, stop=False)
                for c in range(KC):
                    nc.tensor.matmul(fcp[:], hv[:, c * 128:(c + 1) * 128],
                                     W["wfc"][:, c * 2 * NZ:(c + 1) * 2 * NZ],
                                     start=False, stop=(c == KC - 1))
                fc = sp.tile([128, 2 * NZ], F32, tag="fc")
                nc.scalar.copy(fc[:], fcp[:])
                nc.sync.dma_start(out_ap[:, :], fc[:])


def _host_prep(types, params, adj, gt_wi, gt_wh, gt_bi, gt_bh,
               gp_wi, gp_wh, gp_bi, gp_bh, gate_w, gate_b, mapper_w,
               fc1_w, fc1_b, fc2_w, fc2_b):
    """Pure layout prep: transposes/reshapes/one-hot + per-core sharding."""
    f = np.float32

    def chunked(a):  # [512, X] -> [128, 4*X] with K-chunks side by side
        X = a.shape[1]
        return np.ascontiguousarray(
            a.reshape(KC, 128, X).transpose(1, 0, 2).reshape(128, KC * X)).astype(f)

    def fmt(a):  # [B, NV, 512] batch-major -> per-core list of [128, NV*512] fm
        outs = []
        for c in range(NCORES):
            x = a[c * P:(c + 1) * P].reshape(P, NV, KC, 128)
            outs.append(np.ascontiguousarray(
                x.transpose(3, 1, 2, 0).reshape(128, NV * HS)).astype(f))
        return outs

    b1 = np.concatenate([(gt_bi + gt_bh)[:2 * HS], gt_bi[2 * HS:]])
    b2 = np.concatenate([(gp_bi + gp_bh)[:2 * HS], gp_bi[2 * HS:]])
    oh_full = (types[:, :, None] == np.arange(NVT)[None, None, :]).astype(f)
    xni_t_all = fmt(oh_full @ gt_wi[2 * HS:].T + gt_bi[2 * HS:])
    xni_p_all = fmt(params.astype(f) @ gp_wi[2 * HS:].T + gp_bi[2 * HS:])
    bhnf = np.stack([gt_bh[2 * HS:].reshape(KC, 128).T,
                     gp_bh[2 * HS:].reshape(KC, 128).T], 1).reshape(128, 2 * KC)
    shared = {
        "wht_t": chunked(gt_wh.T.astype(f)),
        "wht_p": chunked(gp_wh.T.astype(f)),
        "w1x": np.concatenate([gt_wi.T, b1[None, :]], 0).astype(f),
        "w2x": np.concatenate([gp_wi.T, b2[None, :]], 0).astype(f),
        "bhnf": np.ascontiguousarray(bhnf).astype(f),
        "wgm": chunked(np.concatenate([gate_w[:, :HS].T, mapper_w[:, :HS].T], 1)),
        "bgm": np.stack([np.concatenate([gate_b + gate_w[:, HS + v],
                                         mapper_w[:, HS + v]])
                         for v in range(NV)]).astype(f),
        "vsel": np.repeat(np.eye(NV, dtype=f), P, axis=1),
        "wfc": chunked(np.concatenate([fc1_w.T, fc2_w.T], 1).astype(f)),
        "bfc": np.concatenate([fc1_b, fc2_b])[None, :].astype(f),
        "eye": np.eye(128, dtype=f),
        "ones1": np.ones((1, 128), f),
    }
    oh = (types[:, :, None] == np.arange(NVT)[None, None, :]).astype(f)  # [B,NV,NVT]
    in_maps = []
    for c in range(NCORES):
        s = slice(c * P, (c + 1) * P)
        xt = oh[s].transpose(2, 1, 0).reshape(NVT, NV * P)           # [16, NV*P]
        xt1 = np.concatenate([xt, np.ones((1, NV * P), f)], 0)
        xp = params[s].transpose(2, 1, 0).reshape(FS, NV * P).astype(f)
        xp1 = np.concatenate([xp, np.ones((1, NV * P), f)], 0)
        m = dict(shared)
        m["xt1"] = np.ascontiguousarray(xt1)
        m["xp1"] = np.ascontiguousarray(xp1)
        m["xni_t"] = xni_t_all[c]
        m["xni_p"] = xni_p_all[c]
        m["adjt"] = np.ascontiguousarray(adj[s].reshape(P, NV * NV)).astype(f)
        in_maps.append(m)
    return in_maps


_NC_CACHE = {}


def _get_nc():
    key = str(MMDT)
    if key not in _NC_CACHE:
        _NC_CACHE[key] = build_bass()
    return _NC_CACHE[key]


F32_INPUTS = {"adjt", "eye", "bhnf"}


def kernel(**inputs):
    np_inputs = {k: np.asarray(v) for k, v in inputs.items()}
    in_maps = _host_prep(**np_inputs)
    npdt = mybir.dt.np(MMDT)
    if npdt != np.float32:
        in_maps = [{k: (v if k in F32_INPUTS else v.astype(npdt))
                    for k, v in m.items()} for m in in_maps]
    nc = _get_nc()
    res = run_bass_kernel_spmd(nc, in_maps, core_ids=list(range(NCORES)),
                               **_RUN_KWARGS)
    out = np.concatenate([res.results[c]["out"] for c in range(NCORES)], 0)
    _LAST_RESULT.clear()
    _LAST_RESULT.append(res)
    return out[:, :NZ], out[:, NZ:]


# test.py can set these to enable tracing / inspect results
_RUN_KWARGS = {}
_LAST_RESULT = []

